# revision 41
# baseline (speedup 1.0000x reference)
"""DenseCL contrastive loss on 8 Trainium2 NeuronCores (Bass/Tile).

Strategy: data-parallel over batch B=128 -> 16 batches/core for the dense
heads. The tiny global (pooled) head runs on the HOST in f32/f64 (2% of
the FLOPs) which removes two collectives (g-AllGather + z-AllReduce).

Collective plan: the normalized k dense features (f2) are AllGathered
(400KB f32) right after the k branch, fully hidden under the q branch.
After the local argmax, only the 12.5KB of matched-key INDICES are
AllGathered; each core then re-gathers all 6272 matched keys locally
(8 x ap_gather from the per-core f2 slices - chunk r of the logits rhs
only reads core r's slice, so the local wrapped indices work as-is).
Nothing heavier than 12.5KB sits between the argmax and the logits.

Perf layout:
  - dense-head matmuls run in fp8e4 with DoubleRow perf mode (256-deep
    contraction per pass). X and W are host pre-laid into the exact SBUF
    streaming layout; X loads as ~2 big DMAs and W1 as 5 grouped DMAs
    per branch so HWDGE per-transfer overhead stays off the critical
    path.
  - relu of the hidden dense activations runs on the DVE (tensor_scalar
    add+max), keeping the ACT engine free: the logits phase is bound by
    the exp stream (49 x [128,784] on ACT), with the per-pixel sums
    accumulating on the PE as ones-matmuls lagged two blocks behind so
    the in-order PE queue never parks on an unfinished exp.
  - sim/argmax runs batch-PAIRED: even batch on PSUM partitions 0:64,
    odd on 64:113 via PE tile_position, halving the DVE max/max_index
    chain; the batch un-pairing rides the index broadcast matmul.
  - l2norms, argmax, gather stay f32; lse finishes as ln(rowsum) on the
    host from the single [1,788] output (784 row-sums + pos partial).
"""

import sys

sys.path.insert(0, "/opt/trn_rl_repo")

import numpy as np
import ml_dtypes

import concourse.bacc as bacc
import concourse.mybir as mybir
import concourse.bass_isa as bass_isa
import concourse.tile as tile
from concourse.bass_utils import run_bass_kernel_spmd

dt = mybir.dt
AF = mybir.ActivationFunctionType
DR = mybir.MatmulPerfMode.DoubleRow
ALU = mybir.AluOpType

N_CORES = 8
B, H, W, C = 128, 7, 7, 2048
DH, DE = 2048, 128
HW = H * W                      # 49
BL = B // N_CORES               # 16 batches per core
PIX = BL * HW                   # 784 pixels per core
GPIX = B * HW                   # 6272 global rows
NBLK = GPIX // 128              # 49 column blocks in the logits phase
NPAIR = (NBLK + 1) // 2         # 25 DoubleRow ones-sum pairs (last half-empty)
TAU_INV = 5.0
KC = C // 128                   # 16 contraction chunks
KC2 = KC // 2                   # 8 DoubleRow pairs
MC = DH // 128                  # 16 hidden chunks
MC2 = MC // 2                   # 8 DoubleRow pairs

_NC = None
DEBUG = False

# W1 streaming groups: (first m-chunk, n chunks, pool tag, bufs)
W1_GROUPS = [(0, 1, "w1s", 2), (1, 3, "w1m", 2), (4, 4, "w1l", 3),
             (8, 4, "w1l", 3), (12, 4, "w1l", 3)]


def _build(timing=False, stop=None):
    # timing=True builds a single-core cost-model variant: collectives are
    # skipped and gathered results are read from the local bounce buffer.
    # stop: for cost-model bisection - truncate after a named phase.
    _ph = ["xk", "bkmm", "bk", "bq", "sim", "gather", "rhs", "logits",
           "all"]
    lim = _ph.index(stop) if stop else len(_ph) - 1

    def go(p):
        return _ph.index(p) <= lim

    nc = bacc.Bacc("TRN2", target_bir_lowering=False, debug=False,
                   num_devices=N_CORES)

    def inp(name, shape, d):
        return nc.dram_tensor(name, shape, d, kind="ExternalInput").ap()

    f8 = dt.float8e4
    XSZ = KC2 * 2 * PIX
    W1SZ = MC * KC * 128
    W2SZ = MC * DE
    # merged inputs (fewer per-call args = lighter host dispatch):
    # x8 = [xk | xq]; w8 = [mWd1 | mWd2 | Wd1 | Wd2] in DMA-stream order
    x8 = inp("x8", [128, 2 * XSZ], f8)
    w8 = inp("w8", [128, 2 * (W1SZ + W2SZ)], f8)
    xk8 = x8[:, 0:XSZ]
    xq8 = x8[:, XSZ:2 * XSZ]
    wmd1l = w8[:, 0:W1SZ]
    wmd2l = w8[:, W1SZ:W1SZ + W2SZ]
    wd1l = w8[:, W1SZ + W2SZ:2 * W1SZ + W2SZ]
    wd2l = w8[:, 2 * W1SZ + W2SZ:2 * (W1SZ + W2SZ)]
    # cst: cols 0:41 biases/offsets (col 40 = 49*(p%16) batch offsets);
    # 41:154 identity113 (paired-idx transpose, rows 0:113); 154:282 even
    # / 282:410 odd one-hot batch selection (rows 0:8). The eye/rep part
    # is loaded separately mid-branch, far from the weight-stream head.
    cstall = inp("cstall", [128, 410], dt.float32)
    cst = cstall[:, 0:41]
    csteye = cstall[:, 41:410]
    # single output: cols 0:784 per-pixel exp row-sums, col 784 pos-sum
    rsout = nc.dram_tensor("rowsums", [1, PIX + 4], dt.float32,
                           kind="ExternalOutput").ap()
    if DEBUG:
        didx = nc.dram_tensor("didx", [128, HW], dt.int16,
                              kind="ExternalOutput").ap()

    with tile.TileContext(nc) as tc:
        with (
            tc.tile_pool(name="pers", bufs=1) as pers,
            tc.tile_pool(name="wz", bufs=2) as wz,
            tc.tile_pool(name="work", bufs=2) as work,
            tc.tile_pool(name="dram", bufs=1, space="DRAM") as dram,
        ):
            # ---- constants / biases (DMA deferred into the k branch so
            # the weight/X stream owns the head of the DMA queue) ----
            cstsb = pers.tile([128, 41], dt.float32, name="cstsb")
            csteyesb = pers.tile([128, 369], dt.float32, name="csteyesb")
            onesb = pers.tile([128, 1], dt.float8e4, name="onesb")
            nc.vector.memset(onesb[:], 1.0)
            ballsb = cstsb
            eyesb = csteyesb[:, 0:113]
            repesb = csteyesb[:, 113:241]
            reposb = csteyesb[:, 241:369]
            rssb = pers.tile([1, PIX + 4], dt.float32, name="rssb")
            nc.vector.memset(rssb[:], 0.0)
            _bcols = {"bd1": (0, MC), "mbd1": (16, MC), "bd2": (36, 1),
                      "mbd2": (38, 1)}
            biases = {nm: ballsb[:, c0:c0 + w_]
                      for nm, (c0, w_) in _bcols.items()}
            addsb = ballsb[:, 40:41]

            # X as 2 big fp8 tiles per branch: [128, 8, PIX] chunk-major
            def x_tiles(nm):
                return [pers.tile([128, 4 * 2 * PIX], dt.float8e4,
                                  name=f"{nm}{h}") for h in range(2)]

            def x_dma(t, x_dram, h):
                nc.sync.dma_start(
                    out=t[:], in_=x_dram[:, h * 8 * PIX:(h + 1) * 8 * PIX])

            def xv_pair(xts, k2):
                # rhs view [128, 2, PIX] for DoubleRow chunk pair k2
                j = k2 % 4
                return xts[k2 // 4][:].rearrange(
                    "p (i n) -> p i n", i=8)[:, 2 * j:2 * j + 2, :]

            def norm_cols(z, n, nm, outs, sqtag=None, sqbufs=None):
                """l2-normalize columns of z [128, n] (De on partitions)."""
                sq = work.tile([128, n], dt.float32, tag=sqtag or f"sq{n}",
                               bufs=sqbufs, name=f"sq_{nm}")
                nc.vector.tensor_mul(sq[:], z[:], z[:])
                ssr = work.tile([128, n], dt.float32, tag=f"ssr{n}",
                                name=f"ssr_{nm}")
                nc.gpsimd.partition_all_reduce(ssr[:], sq[:], 128,
                                               bass_isa.ReduceOp.add)
                srt = work.tile([128, n], dt.float32, tag=f"srt{n}",
                                name=f"srt_{nm}")
                nc.scalar.activation(srt[:], ssr[:], AF.Sqrt)
                rr = work.tile([128, n], dt.float32, tag=f"rr{n}",
                               name=f"rr_{nm}")
                nc.vector.reciprocal(rr[:], srt[:])
                for o in outs:
                    nc.vector.tensor_mul(o, z[:], rr[:])
                return rr

            with tc.tile_pool(name="ps", bufs=2, space="PSUM") as ps:

                def dense_branch(xts, w1_dram, b1, w2_dram, b2, nm, inter):
                    """2-layer fp8 DoubleRow head -> ZT [128, 784] f32.

                    inter: {m: [(dst, src), ...]} extra DMAs issued on
                    entering hidden chunk m (keeps the single HWDGE queue
                    fed in the exact order the PE consumes)."""
                    w2sb = wz.tile([128, MC * DE], dt.float8e4, tag="w2sb",
                                   name=f"w2_{nm}")

                    ztp = ps.tile([128, PIX], dt.float32, tag="ztp", bufs=1,
                                  name=f"ztp_{nm}")
                    pairs = []

                    def layer2(m2):
                        # emitted one m-iteration late so the relu feeding
                        # it has already drained from the DVE queue
                        lhs2 = w2sb[:, m2 * 256:(m2 + 1) * 256].rearrange(
                            "p (i m) -> p i m", i=2)
                        hv = pairs[m2][:].rearrange("p (i n) -> p i n", i=2)
                        nc.tensor.matmul(ztp[:, 0:512], lhs2,
                                         hv[:, :, 0:512],
                                         start=(m2 == 0),
                                         stop=(m2 == MC2 - 1),
                                         perf_mode=DR)
                        nc.tensor.matmul(ztp[:, 512:PIX], lhs2,
                                         hv[:, :, 512:PIX],
                                         start=(m2 == 0),
                                         stop=(m2 == MC2 - 1),
                                         perf_mode=DR)

                    gidx = 0
                    cur = None
                    for m in range(MC):
                        if gidx < len(W1_GROUPS) and W1_GROUPS[gidx][0] == m:
                            g0, gn, tag, gb = W1_GROUPS[gidx]
                            gidx += 1
                            gt = wz.tile([128, gn * KC * 128], dt.float8e4,
                                         tag=tag, bufs=gb,
                                         name=f"{tag}_{nm}{g0}")
                            nc.sync.dma_start(
                                out=gt[:],
                                in_=w1_dram[:, g0 * KC * 128:
                                            (g0 + gn) * KC * 128])
                            cur = (gt, g0)
                        for dst, src in inter.get(m, ()):
                            nc.sync.dma_start(out=dst, in_=src)
                        if m == 1:
                            nc.sync.dma_start(out=w2sb[:], in_=w2_dram[:])
                        gt, g0 = cur
                        wcol = gt[:, (m - g0) * KC * 128:
                                  (m - g0 + 1) * KC * 128]
                        h1p = ps.tile([128, PIX], dt.float32, tag="bigp",
                                      name=f"h1p_{nm}{m}")
                        for k2 in range(KC2):
                            lhs = wcol[:, k2 * 256:(k2 + 1) * 256].rearrange(
                                "p (i m) -> p i m", i=2)
                            xv = xv_pair(xts, k2)
                            nc.tensor.matmul(h1p[:, 0:512], lhs,
                                             xv[:, :, 0:512],
                                             start=(k2 == 0),
                                             stop=(k2 == KC2 - 1),
                                             perf_mode=DR)
                            nc.tensor.matmul(h1p[:, 512:PIX], lhs,
                                             xv[:, :, 512:PIX],
                                             start=(k2 == 0),
                                             stop=(k2 == KC2 - 1),
                                             perf_mode=DR)
                        sub = m & 1
                        if sub == 0:
                            pairs.append(
                                work.tile([128, 2 * PIX], dt.float8e4,
                                          tag="h1pair", bufs=3,
                                          name=f"h1_{nm}{m}"))
                        # relu on the DVE: the ACT engine stays free for
                        # sqrt/exp, and the DVE has slack in this phase.
                        # The very last relu is split in two so the final
                        # layer2's first column-group (which gates zq8a and
                        # the whole argmax chain) starts half a relu early.
                        if m == MC - 1:
                            nc.vector.tensor_scalar(
                                pairs[-1][:, PIX:PIX + 512], h1p[:, 0:512],
                                b1[:, m:m + 1], 0.0, ALU.add, ALU.max)
                            nc.vector.tensor_scalar(
                                pairs[-1][:, PIX + 512:2 * PIX],
                                h1p[:, 512:PIX],
                                b1[:, m:m + 1], 0.0, ALU.add, ALU.max)
                        else:
                            nc.vector.tensor_scalar(
                                pairs[-1][:, sub * PIX:(sub + 1) * PIX],
                                h1p[:],
                                b1[:, m:m + 1], 0.0, ALU.add, ALU.max)
                        if m >= 3 and sub == 1:
                            layer2(m // 2 - 1)
                    layer2(MC2 - 1)
                    return ztp

                # ---- load X, momentum branch first ----
                xkts = x_tiles("xk")
                xqts = x_tiles("xq")
                # first X tile in two halves: the m0 weight DMA starts
                # ~1.2us earlier, so the PE starts (and p-state ramps)
                # sooner
                nc.sync.dma_start(out=xkts[0][:, 0:4 * PIX],
                                  in_=xk8[:, 0:4 * PIX])

                def xsrc(x_dram, h):
                    return x_dram[:, h * 8 * PIX:(h + 1) * 8 * PIX]

                if go("bkmm"):
                    ztkp = dense_branch(
                        xkts, wmd1l, biases["mbd1"], wmd2l, biases["mbd2"],
                        "k",
                        {0: [(xkts[0][:, 4 * PIX:8 * PIX],
                              xk8[:, 4 * PIX:8 * PIX]),
                             (xkts[1][:], xsrc(xk8, 1)),
                             (cstsb[:], cst[:])],
                         13: [(xqts[0][:], xsrc(xq8, 0))]})
                if go("bk"):
                    ztk = work.tile([128, PIX], dt.float32, tag="zt",
                                    name="zt_k")
                    nc.vector.tensor_scalar_add(ztk[:], ztkp[:],
                                                biases["mbd2"])
                    f2tb = pers.tile([128, PIX], dt.float8e4, name="f2tb")
                    f2tf = pers.tile([128, PIX], dt.float32, name="f2tf")
                    norm_cols(ztk, PIX, "f2", [f2tb[:], f2tf[:]])
                    # AllGather the normalized k dense features NOW - the
                    # 400KB f32 collective flies entirely under the q dense
                    # branch; after the (tiny) idx AllGather below, every
                    # core re-gathers all 6272 matched keys locally, so no
                    # 800KB matched-key collective sits on the critical path
                    shsp = "Local" if timing else "Shared"
                    ag2in = dram.tile([128, PIX], dt.float32, name="ag2in")
                    ag2out = dram.tile([128 * N_CORES, PIX], dt.float32,
                                       addr_space=shsp, name="ag2out")
                    nc.sync.dma_start(out=ag2in[:], in_=f2tf[:])
                    if not timing:
                        nc.gpsimd.collective_compute(
                            "AllGather", mybir.AluOpType.bypass,
                            replica_groups=[list(range(N_CORES))],
                            ins=[ag2in.opt()], outs=[ag2out.opt()])
                    else:
                        nc.sync.dma_start(out=ag2out[0:128, :],
                                          in_=f2tf[:])

                # ---- query dense branch ----
                if go("bq"):
                    ztqp = dense_branch(
                        xqts, wd1l, biases["bd1"], wd2l, biases["bd2"], "q",
                        {0: [(xqts[1][:], xsrc(xq8, 1))],
                         2: [(csteyesb[:], csteye[:])]})

                # ---- per-batch sim + argmax on the UN-normalized query
                # features (argmax is invariant to the per-pixel positive
                # scale), so the idx -> gather -> AllGather chain never
                # waits on the f1 norm ----
                if go("sim"):
                    # fused bias-add + fp8 convert straight from PSUM, in
                    # two half tiles so the first 8 batches' sim matmuls
                    # start while the second half converts
                    HX = PIX // 2
                    zq8a = work.tile([128, HX], dt.float8e4, tag="zq8a",
                                     name="zq8a")
                    zq8b = work.tile([128, HX], dt.float8e4, tag="zq8b",
                                     name="zq8b")
                    nc.vector.tensor_scalar_add(zq8a[:], ztqp[:, 0:HX],
                                                biases["bd2"])
                    nc.vector.tensor_scalar_add(zq8b[:], ztqp[:, HX:PIX],
                                                biases["bd2"])
                    # batch-PAIRED sims: even batch 2j lands on partitions
                    # 0:64 (stationary widened to 64 cols - the 15 extra
                    # rows are harmless finite garbage), odd batch 2j+1 on
                    # partitions 64:113 via PE tile_position, so one
                    # max/max_index pass covers TWO batches
                    simall = ps.tile([128, 8 * HW], dt.float32,
                                     tag="smallp", name="simall")
                    for j in range(BL // 2):
                        e, o = 2 * j, 2 * j + 1
                        zhe = zq8a if e < 8 else zq8b
                        zho = zq8a if o < 8 else zq8b
                        ce = (e % 8) * HW
                        co = (o % 8) * HW
                        nc.tensor.matmul(simall[0:64, j * HW:(j + 1) * HW],
                                         zhe[:, ce:ce + 64],
                                         f2tb[:, e * HW:(e + 1) * HW],
                                         start=True, stop=True)
                        nc.tensor.matmul(
                            simall[64:113, j * HW:(j + 1) * HW],
                            zho[:, co:co + HW],
                            f2tb[:, o * HW:(o + 1) * HW],
                            start=True, stop=True)
                    # f32 biased q output for the f1 norm / pos path; runs
                    # on the DVE while the PE does the sim matmuls
                    ztq = work.tile([128, PIX], dt.float32, tag="zt",
                                    name="zt_q")
                    nc.vector.tensor_scalar_add(ztq[:], ztqp[:],
                                                biases["bd2"])
                    simsb = work.tile([128, 8 * HW], dt.float32,
                                      tag="mid784", bufs=1, name="simsb")
                    # PSUM->SBUF hop on the ACT engine (table-free Copy):
                    # the DVE is the serial resource in this stretch
                    nc.scalar.activation(simsb[0:113, :], simall[0:113, :],
                                         AF.Copy)
                    mi2 = pers.tile([128, 64], dt.uint16, name="mi2")
                    for j in range(BL // 2):
                        mx8 = work.tile([128, 8], dt.float32, tag="mx8",
                                        name=f"mx{j}")
                        nc.vector.max(mx8[0:113, :],
                                      simsb[0:113, j * HW:(j + 1) * HW])
                        nc.vector.max_index(mi2[0:113, j * 8:j * 8 + 8],
                                            mx8[0:113, :],
                                            simsb[0:113, j * HW:(j + 1) * HW])
                    idxcp = pers.tile([128, 8], dt.float32, name="idxcp")
                    nc.vector.tensor_copy(
                        idxcp[0:113, 0:8].rearrange("p (j e) -> p j e",
                                                    e=1),
                        mi2[:].rearrange(
                            "p (j e) -> p j e", e=8)[0:113, :, 0:1])

                # ---- wrapped gather indices, gather, AllGather ----
                if go("gather"):
                    tpp = ps.tile([16, 128], dt.float32, tag="smallp",
                                  name="tpp")
                    nc.tensor.transpose(tpp[0:8, 0:113], idxcp[0:113, 0:8],
                                        eyesb[0:113, 0:113])
                    idxf = work.tile([16, 128], dt.float32, tag="idxf",
                                     bufs=1, name="idxf")
                    nc.vector.tensor_copy(idxf[0:8, 0:113],
                                          tpp[0:8, 0:113])
                    # un-pair to all 128 partitions with two one-hot
                    # selection matmuls: even batches from cols 0:49, odd
                    # from cols 64:113; then add the per-partition batch
                    # offset 49*(p%16) during the int16 convert
                    idxp = ps.tile([128, B], dt.float32, tag="smallp",
                                   name="idxp")
                    nc.tensor.matmul(idxp[:, 0:HW], repesb[0:8, :],
                                     idxf[0:8, 0:HW], start=True,
                                     stop=False)
                    nc.tensor.matmul(idxp[:, 0:HW], reposb[0:8, :],
                                     idxf[0:8, 64:64 + HW], start=False,
                                     stop=True)
                    idxr = pers.tile([128, HW], dt.int16, name="idxr")
                    nc.vector.tensor_scalar_add(idxr[:], idxp[:, 0:HW],
                                                addsb)
                    # tiny (12.5KB) idx AllGather: chunk r of the logits
                    # rhs only ever reads core r's f2 slice, so the LOCAL
                    # wrapped indices are exchanged as-is - no offsets
                    ag3in = dram.tile([128, HW], dt.int16, name="ag3in")
                    ag3out = dram.tile([128 * N_CORES, HW], dt.int16,
                                       addr_space=shsp, name="ag3out")
                    nc.sync.dma_start(out=ag3in[:], in_=idxr[:])
                    if not timing:
                        nc.gpsimd.collective_compute(
                            "AllGather", mybir.AluOpType.bypass,
                            replica_groups=[list(range(N_CORES))],
                            ins=[ag3in.opt()], outs=[ag3out.opt()])
                    else:
                        nc.sync.dma_start(out=ag3out[0:128, :],
                                          in_=idxr[:])
                    # f1 norm overlaps the AllGather; only the fp8 output
                    # is on the logits path - the f32 copy (pos-only) and
                    # the matched-key gather (pos-only) are deferred to the
                    # rhs section so they never block this chain
                    f1tb = pers.tile([128, PIX], dt.float8e4, name="f1tb")
                    f1tf = pers.tile([128, PIX], dt.float32, name="f1tf")
                    # sq shares the bufs=1 "idxf" slot: the f1 norm's DVE
                    # work is forced AFTER idxp has consumed idxf, keeping
                    # the argmax->idx->AllGather DVE chain clear (the norm
                    # has slack until the first logits block)
                    f1rr = norm_cols(ztq, PIX, "f1", [f1tb[:]],
                                     sqtag="idxf", sqbufs=1)
                # ---- rhs assembly: DMA each core's f2 slice, gather its
                # matched keys with its (gathered) indices, convert to fp8.
                # The slice DMAs share the zq8a/zq8b pool tags so they are
                # forced AFTER the sim matmuls in the DMA queue - keeping
                # the 3.2MB of slice traffic out of the q weight stream.
                if go("rhs"):
                    idxga = pers.tile([128, N_CORES * HW], dt.int16,
                                      name="idxga")
                    nc.sync.dma_start(
                        out=idxga[:].rearrange("p (r n) -> p r n", n=HW),
                        in_=ag3out[:].rearrange("(r q) n -> q r n", q=128))
                    rhsb = pers.tile([128, GPIX], dt.float8e4, name="rhsb")
                    for r in range(N_CORES):
                        f2sl = work.tile([128, PIX], dt.float32,
                                         tag=("zq8a", "zq8b")[r % 2],
                                         name=f"f2sl{r}")
                        nc.sync.dma_start(
                            out=f2sl[:],
                            in_=ag2out[r * 128:(r + 1) * 128, :])
                        rhsf = work.tile([128, PIX], dt.float32,
                                         tag="rhsf", bufs=3,
                                         name=f"rhsf{r}")
                        if r == 0:
                            # split the logits-gating first gather so the
                            # first key blocks land ~0.6us earlier (idx
                            # splits must stay multiples of the 16-wrap)
                            NIA = 24 * 16
                            nc.gpsimd.ap_gather(
                                rhsf[:, 0:NIA], f2sl[:],
                                idxga[:, 0:24],
                                channels=128, num_elems=PIX, d=1,
                                num_idxs=NIA)
                            nc.vector.tensor_copy(rhsb[:, 0:NIA],
                                                  rhsf[:, 0:NIA])
                            nc.gpsimd.ap_gather(
                                rhsf[:, NIA:PIX], f2sl[:],
                                idxga[:, 24:HW],
                                channels=128, num_elems=PIX, d=1,
                                num_idxs=PIX - NIA)
                            nc.vector.tensor_copy(rhsb[:, NIA:PIX],
                                                  rhsf[:, NIA:PIX])
                            continue
                        nc.gpsimd.ap_gather(
                            rhsf[:], f2sl[:],
                            idxga[:, r * HW:(r + 1) * HW],
                            channels=128, num_elems=PIX, d=1, num_idxs=PIX)
                        nc.vector.tensor_copy(
                            rhsb[:, r * PIX:(r + 1) * PIX], rhsf[:])
                    # pos = per-pixel dot f1 . matched. ap_gather's 16-way
                    # index wrap means mtf column j holds the match for
                    # (batch j%16, pixel j//16) - pair via permuted views.
                    # Emitted after the rhs gathers (pmul shares the rhsf
                    # tag) so the pos chain's Pool/DVE work never delays
                    # the logits-gating gathers - it has ~40us of slack.
                    nc.vector.tensor_mul(f1tf[:], ztq[:], f1rr[:])
                    mtf = pers.tile([128, PIX], dt.float32, name="mtf")
                    nc.gpsimd.ap_gather(mtf[:], f2tf[:], idxr[:],
                                        channels=128, num_elems=PIX, d=1,
                                        num_idxs=PIX)
                    pmul = work.tile([128, PIX], dt.float32, tag="rhsf",
                                     bufs=3, name="pmul")
                    nc.vector.tensor_mul(
                        pmul[:].rearrange("p (w b) -> p w b", b=BL),
                        f1tf[:].rearrange("p (b w) -> p w b", w=HW),
                        mtf[:].rearrange("p (w b) -> p w b", b=BL))
                    ppar = work.tile([128, PIX], dt.float32, tag="ppar",
                                     name="ppar")
                    nc.gpsimd.partition_all_reduce(ppar[:], pmul[:], 128,
                                                   bass_isa.ReduceOp.add)
                    nc.vector.tensor_reduce(rssb[0:1, PIX:PIX + 1],
                                            ppar[0:1, :],
                                            axis=mybir.AxisListType.X,
                                            op=mybir.AluOpType.add)

                # ---- logits phase (same PSUM pool: a fresh pool would
                # insert a full-engine barrier right where the Exp table
                # load needs to prefetch) ----
                if go("logits"):
                    # per-pixel sum of exp over all 6272 matched keys,
                    # accumulated across the 49 blocks on the PE via plain
                    # fp8 ones-matmuls (the phase is exp/ACT-bound, so the
                    # extra PE cycles vs DoubleRow are free)
                    rowsum = ps.tile([1, PIX], dt.float32, tag="ztp",
                                     bufs=1, name="rowsum")
                    exps = []

                    def ones_sum(blk):
                        nc.tensor.matmul(rowsum[0:1, 0:512], onesb[:],
                                         exps[blk][:, 0:512],
                                         start=(blk == 0),
                                         stop=(blk == NBLK - 1))
                        nc.tensor.matmul(rowsum[0:1, 512:PIX], onesb[:],
                                         exps[blk][:, 512:PIX],
                                         start=(blk == 0),
                                         stop=(blk == NBLK - 1))

                    for blk in range(NBLK):
                        lhs = rhsb[:, blk * 128:(blk + 1) * 128]
                        lpt = ps.tile([128, PIX], dt.float32, tag="bigp",
                                      name=f"lpt{blk}")
                        nc.tensor.matmul(lpt[:, 0:512], lhs,
                                         f1tb[:, 0:512],
                                         start=True, stop=True)
                        nc.tensor.matmul(lpt[:, 512:PIX], lhs,
                                         f1tb[:, 512:PIX],
                                         start=True, stop=True)
                        exps.append(work.tile([128, PIX], dt.float8e4,
                                              tag="expsb", bufs=6,
                                              name=f"ex{blk}"))
                        nc.scalar.activation(exps[-1][:], lpt[:], AF.Exp,
                                             scale=TAU_INV)
                        # software pipeline, lag TWO blocks: with lag one,
                        # the in-order PE queue parks ones_sum(b-1) on the
                        # not-yet-finished exp(b-1), blocking lpt(b+1) and
                        # rippling a ~230ns/block bubble into the ACT stream
                        if blk > 1:
                            ones_sum(blk - 2)
                    ones_sum(NBLK - 2)
                    ones_sum(NBLK - 1)

                if go("all"):
                    # per-pixel exp sums go to the host, which finishes
                    # lse = ln(rowsum) there (saves an ACT Ln-table load
                    # plus a reduce on the device's critical tail)
                    nc.scalar.activation(rssb[:, 0:PIX], rowsum[0:1, :],
                                         AF.Copy)
                    nc.sync.dma_start(out=rsout[:], in_=rssb[:])
                    if DEBUG:
                        nc.sync.dma_start(out=didx[:], in_=idxr[:])

    nc.compile()
    return nc


def _get_nc():
    global _NC
    if _NC is None:
        _NC = _build()
    return _NC


def _prep_inputs(inputs):
    e4 = ml_dtypes.float8_e4m3
    f32 = np.float32

    def w1_layout(w):
        # [C, DH] -> [128, MC*KC*128]: A[p, m, k, j] = W[k*128+p, m*128+j]
        a = np.asarray(w, f32).astype(e4).reshape(KC, 128, MC, 128)
        return np.ascontiguousarray(
            a.transpose(1, 2, 0, 3).reshape(128, MC * KC * 128))

    def w2_layout(w):
        # [DH, DE] -> [128, MC*DE]: A[p, m, j] = W[m*128+p, j]
        a = np.asarray(w, f32).astype(e4).reshape(MC, 128, DE)
        return np.ascontiguousarray(
            a.transpose(1, 0, 2).reshape(128, MC * DE))

    def x_layout(x):
        # [PIX, C] -> [128, KC2*2*PIX]: A[p, k2, i, n] = X[n, (2*k2+i)*128+p]
        a = np.ascontiguousarray(x.T).astype(e4).reshape(KC2, 2, 128, PIX)
        return np.ascontiguousarray(
            a.transpose(2, 0, 1, 3).reshape(128, KC2 * 2 * PIX))

    def b1(v, mc):
        return np.ascontiguousarray(np.asarray(v, f32).reshape(mc, 128).T)

    def b2(v):
        return np.ascontiguousarray(np.asarray(v, f32).reshape(128, 1))

    ball0 = np.zeros((128, 41), f32)
    ball0[:, 0:MC] = b1(inputs["bd1"], MC)
    ball0[:, 16:16 + MC] = b1(inputs["mbd1"], MC)
    ball0[:, 36] = b2(inputs["bd2"])[:, 0]
    ball0[:, 38] = b2(inputs["mbd2"])[:, 0]
    ball0[:, 40] = HW * (np.arange(128) % BL).astype(f32)
    eye0 = np.zeros((128, 369), f32)
    eye0[0:113, 0:113] = np.eye(113, dtype=f32)
    p = np.arange(128)
    for j in range(8):
        eye0[j, 113:241] = (p % BL == 2 * j).astype(f32)
        eye0[j, 241:369] = (p % BL == 2 * j + 1).astype(f32)
    common = {
        "w8": np.concatenate(
            [w1_layout(inputs["mWd1"]), w2_layout(inputs["mWd2"]),
             w1_layout(inputs["Wd1"]), w2_layout(inputs["Wd2"])], axis=1),
        "cstall": np.ascontiguousarray(
            np.concatenate([ball0, eye0], axis=1)),
    }
    fq = np.asarray(inputs["feat_q"], f32).reshape(B, HW, C)
    fk = np.asarray(inputs["feat_k"], f32).reshape(B, HW, C)
    in_maps = []
    for r in range(N_CORES):
        sl = slice(r * BL, (r + 1) * BL)
        m = dict(common)
        m["x8"] = np.concatenate(
            [x_layout(fk[sl].reshape(PIX, C)),
             x_layout(fq[sl].reshape(PIX, C))], axis=1)
        in_maps.append(m)
    return in_maps


def _host_lg(inputs):
    """Global (pooled) head InfoNCE, fully on host (~2% of the FLOPs)."""
    f32 = np.float32
    gq = np.asarray(inputs["feat_q"], f32).mean(axis=(1, 2))
    gk = np.asarray(inputs["feat_k"], f32).mean(axis=(1, 2))
    zq = np.maximum(gq @ np.asarray(inputs["Wg1"], f32)
                    + np.asarray(inputs["bg1"], f32), 0.0) \
        @ np.asarray(inputs["Wg2"], f32) + np.asarray(inputs["bg2"], f32)
    zk = np.maximum(gk @ np.asarray(inputs["mWg1"], f32)
                    + np.asarray(inputs["mbg1"], f32), 0.0) \
        @ np.asarray(inputs["mWg2"], f32) + np.asarray(inputs["mbg2"], f32)

    def l2n_rows(z):
        z = np.asarray(z, np.float64)
        return z / np.sqrt(np.maximum((z * z).sum(axis=1, keepdims=True),
                                      1e-12))

    logits = (l2n_rows(zq) @ l2n_rows(zk).T) * TAU_INV
    mx = logits.max(axis=1)
    lse = mx + np.log(np.exp(logits - mx[:, None]).sum(axis=1))
    return float(np.mean(lse - np.diagonal(logits)))


def _combine(results, inputs):
    sld = smd = 0.0
    for r in range(N_CORES):
        rs = np.asarray(results[r]["rowsums"], np.float64).reshape(-1)
        sld += float(np.log(rs[0:PIX]).sum())
        smd += rs[PIX]
    l_d = (sld - TAU_INV * smd) / GPIX
    l_g = _host_lg(inputs)
    return np.float32(0.5 * l_g + 0.5 * l_d)


def kernel(**inputs) -> np.ndarray:
    nc = _get_nc()
    in_maps = _prep_inputs(inputs)
    res = run_bass_kernel_spmd(nc, in_maps, list(range(N_CORES)))
    return np.asarray(_combine(res.results, inputs))


if __name__ == "__main__":
    import jax
    import reference

    with jax.default_device(jax.devices("cpu")[0]):
        inputs = {k: np.asarray(v)
                  for k, v in reference.setup_inputs().items()}
        exp = np.asarray(reference.reference(**reference.setup_inputs()))
    got = kernel(**inputs)
    print("got", got, "exp", exp, "relerr", abs(got / exp - 1.0))
